# revision 8
# baseline (speedup 1.0000x reference)
"""Trainium2 Bass kernel for nn_AttentionHead_37469294691171.

Single attention head with softmax over the QUERY axis (dim=1 of the
[B, T(query), S(key)] score tensor), causal mask, B=8, T=4096, E=2048, D=128.

Strategy: pure data-parallel over batch — one batch element per NeuronCore,
no collectives.  Per core, everything is computed in the *transposed* score
layout W[s, t] = (k_s . q_t) * scale so the query-axis softmax becomes a
free-axis reduction:

  1. host pre-transposes x[b] -> xT [E, T] and casts inputs to bf16
  2. qT/kT/vT [D, T] projections via PSUM-accumulated matmuls over E-chunks
     (softmax scale folded into qT, v transposed back to natural [s, d])
  3. for each 128-row s-tile j and 512-col t-block b >= diag: one matmul
     gives W tile [s=128, t=512]; additive -30000 mask on the diagonal
     128x128 sub-tile; ScalarE exp with fused accum_out produces the
     unnormalized P tile (kept resident in SBUF, bf16) and the per-key
     partial sums of the softmax denominator in one pass
  4. l = sum of partials, v' = v * (1/l) folded per-partition into v
  5. outT[d, t-block] = sum_j v'_j^T @ P_j  (PSUM-accumulated, N=512)
  6. host transposes outT back

Only the lower causal triangle is ever computed (both for scores and for
the output contraction).
"""

import numpy as np
import ml_dtypes

B, T, E, D = 8, 4096, 2048, 128
NBLK = 512            # t-block width
NCHUNK = E // 128     # 16 contraction chunks for the projections
NB = T // NBLK        # 8 t-blocks
NJ = T // 128         # 32 s-tiles
SCALE = float(D) ** -0.5
NEG = -30000.0        # additive mask; exp(-30000) == 0 in f32

_CACHE = {}


def _split_multiwaits(bir: bytes) -> bytes:
    """walrus accepts at most ONE embedded sync-wait per engine instruction.

    Tile's scheduler attaches every required wait to the consuming
    instruction's sync_info, which trips "Too many sync wait commands" in
    walrus codegen whenever an instruction joins events from 2+ engines.
    Hoist all but the last wait onto wait-only EventSemaphore instructions
    (the exact representation `engine.wait_ge()` uses in raw bass) placed
    immediately before the instruction in the same engine stream.  Each
    engine executes its stream in the Tile-scheduled order and waits only
    reference earlier-scheduled events, so this preserves semantics.
    """
    import json

    j = json.loads(bir)
    n = 0
    for fn in j["functions"]:
        for bb in fn["blocks"]:
            out = []
            for inst in bb["instructions"]:
                si = inst.get("sync_info")
                waits = (si or {}).get("on_wait") or []
                if len(waits) > 1 and inst.get("opcode") != "ISA":
                    for w in waits[:-1]:
                        n += 1
                        out.append({
                            "debug": inst.get("debug", 0),
                            "engine": inst["engine"],
                            "ins": [],
                            "outs": [],
                            "name": f"{inst['name']}-hw{n}",
                            "opcode": "EventSemaphore",
                            "sync_info": {"on_update": [], "on_wait": [w]},
                        })
                    si["on_wait"] = [waits[-1]]
                out.append(inst)
            bb["instructions"] = out
    return json.dumps(j).encode()


def _build():
    import concourse.bass as bass
    import concourse.mybir as mybir
    from concourse.tile import TileContext
    from concourse.masks import make_identity

    f32 = mybir.dt.float32
    bf16 = mybir.dt.bfloat16
    Alu = mybir.AluOpType

    nc = bass.Bass()
    xT_d = nc.declare_dram_parameter("xT", [E, T], bf16, isOutput=False)
    wq_d = nc.declare_dram_parameter("wq", [E, D], bf16, isOutput=False)
    wk_d = nc.declare_dram_parameter("wk", [E, D], bf16, isOutput=False)
    wv_d = nc.declare_dram_parameter("wv", [E, D], bf16, isOutput=False)
    out_d = nc.declare_dram_parameter("out", [D, T], f32, isOutput=True)

    with TileContext(nc) as tc:
        with (
            tc.tile_pool(name="const", bufs=1) as constp,
            tc.tile_pool(name="xtb", bufs=2) as xtbp,
            tc.tile_pool(name="big", bufs=1) as bigp,
            tc.tile_pool(name="vtb", bufs=2) as vtbp,
            tc.tile_pool(name="pstore", bufs=1) as pp,
            tc.tile_pool(name="outp", bufs=1) as outp,
            tc.tile_pool(name="ps_qkv", bufs=2, space="PSUM") as ps_qkv,
            tc.tile_pool(name="ps_vt", bufs=2, space="PSUM") as ps_vt,
            tc.tile_pool(name="ps_w", bufs=3, space="PSUM") as ps_w,
            tc.tile_pool(name="ps_o", bufs=1, space="PSUM") as ps_o,
        ):
            ident = constp.tile([128, 128], bf16)
            make_identity(nc, ident[:])

            # trimask[p, c] = 0 where c >= p else NEG  (allowed: t >= s)
            trimask = constp.tile([128, 128], f32)
            nc.gpsimd.memset(trimask[:], 0.0)
            nc.gpsimd.affine_select(
                out=trimask[:],
                in_=trimask[:],
                compare_op=Alu.is_ge,
                fill=NEG,
                base=0,
                pattern=[[1, 128]],
                channel_multiplier=-1,
            )

            w_sb = {}
            for nm, wd in (("wq", wq_d), ("wk", wk_d), ("wv", wv_d)):
                t_ = constp.tile([128, NCHUNK, D], bf16, tag=f"w_{nm}")
                for c in range(NCHUNK):
                    nc.sync.dma_start(t_[:, c, :], wd[c * 128:(c + 1) * 128, :])
                w_sb[nm] = t_

            qT = bigp.tile([128, T], bf16, tag="qT")
            kT = bigp.tile([128, T], bf16, tag="kT")
            v_nat = bigp.tile([128, T], bf16, tag="v")  # 32 tiles of [s=128, d=128]
            lpart = bigp.tile([128, NJ, NB], f32, tag="lpart")
            lsum = bigp.tile([128, NJ], f32, tag="lsum")
            linv = bigp.tile([128, NJ], f32, tag="linv")
            nc.vector.memset(lpart[:], 0.0)

            ptiles = [
                pp.tile([128, T - 128 * j], bf16, tag=f"P{j}", name=f"P{j}")
                for j in range(NJ)
            ]

            for b in range(NB):
                bs = slice(b * NBLK, (b + 1) * NBLK)
                xtb = xtbp.tile([128, NCHUNK, NBLK], bf16, tag="xtb")
                for c in range(NCHUNK):
                    nc.sync.dma_start(xtb[:, c, :], xT_d[c * 128:(c + 1) * 128, bs])

                # ---- projections for this t-block ----
                psq = ps_qkv.tile([128, NBLK], f32, tag="qkv")
                for c in range(NCHUNK):
                    nc.tensor.matmul(psq[:], lhsT=w_sb["wq"][:, c, :], rhs=xtb[:, c, :],
                                     start=(c == 0), stop=(c == NCHUNK - 1))
                nc.vector.tensor_scalar_mul(qT[:, bs], psq[:], SCALE)

                psk = ps_qkv.tile([128, NBLK], f32, tag="qkv")
                for c in range(NCHUNK):
                    nc.tensor.matmul(psk[:], lhsT=w_sb["wk"][:, c, :], rhs=xtb[:, c, :],
                                     start=(c == 0), stop=(c == NCHUNK - 1))
                nc.vector.tensor_copy(kT[:, bs], psk[:])

                psv = ps_qkv.tile([128, NBLK], f32, tag="qkv")
                for c in range(NCHUNK):
                    nc.tensor.matmul(psv[:], lhsT=w_sb["wv"][:, c, :], rhs=xtb[:, c, :],
                                     start=(c == 0), stop=(c == NCHUNK - 1))
                vtb = vtbp.tile([128, NBLK], bf16, tag="vtb")
                nc.vector.tensor_copy(vtb[:], psv[:])
                for i in range(4):
                    pst = ps_vt.tile([128, 128], bf16, tag="vt")
                    nc.tensor.transpose(pst[:], vtb[:, i * 128:(i + 1) * 128], ident[:])
                    j = 4 * b + i
                    nc.vector.tensor_copy(v_nat[:, j * 128:(j + 1) * 128], pst[:])

                # ---- score stats rows: all s-tiles j with any allowed cols here ----
                for j in range(4 * b + 4):
                    psw = ps_w.tile([128, NBLK], f32, tag="w")
                    nc.tensor.matmul(psw[:], lhsT=kT[:, j * 128:(j + 1) * 128],
                                     rhs=qT[:, bs], start=True, stop=True)
                    if j >= 4 * b:
                        # diagonal 128x128 sub-tile needs the causal mask
                        off = (j - 4 * b) * 128
                        nc.vector.tensor_tensor(psw[:, off:off + 128],
                                                psw[:, off:off + 128],
                                                trimask[:], Alu.add)
                        src = psw[:, off:NBLK]
                        dst = ptiles[j][:, 0:NBLK - off]
                    else:
                        c0 = b * NBLK - 128 * j
                        src = psw[:]
                        dst = ptiles[j][:, c0:c0 + NBLK]
                    nc.scalar.activation(dst, src,
                                         mybir.ActivationFunctionType.Exp,
                                         accum_out=lpart[:, j, b:b + 1])

            # ---- softmax denominators; fold 1/l into v ----
            nc.vector.tensor_reduce(lsum[:], lpart[:], axis=mybir.AxisListType.X,
                                    op=mybir.AluOpType.add)
            nc.vector.reciprocal(linv[:], lsum[:])
            for j in range(NJ):
                nc.vector.tensor_scalar(v_nat[:, j * 128:(j + 1) * 128],
                                        v_nat[:, j * 128:(j + 1) * 128],
                                        linv[:, j:j + 1], None, Alu.mult)

            # ---- output: outT[d, t-block] = sum_j v'_j^T @ P_j ----
            for b in range(NB):
                bs = slice(b * NBLK, (b + 1) * NBLK)
                pso = ps_o.tile([128, NBLK], f32, tag="o")
                nmm = 4 * b + 4
                for j in range(nmm):
                    vpj = v_nat[:, j * 128:(j + 1) * 128]
                    last = (j == nmm - 1)
                    if j >= 4 * b:
                        off = (j - 4 * b) * 128
                        nc.tensor.matmul(pso[:, off:NBLK], lhsT=vpj,
                                         rhs=ptiles[j][:, 0:NBLK - off],
                                         start=(j == 0), stop=last)
                    else:
                        c0 = b * NBLK - 128 * j
                        nc.tensor.matmul(pso[:], lhsT=vpj,
                                         rhs=ptiles[j][:, c0:c0 + NBLK],
                                         start=(j == 0), stop=last)
                ob = outp.tile([128, NBLK], f32, tag="ob")
                nc.vector.tensor_copy(ob[:], pso[:])
                nc.sync.dma_start(out_d[:, bs], ob[:])

    return nc


def _get_nc():
    if "nc" not in _CACHE:
        nc = _build()
        orig = nc.to_json_bytes
        nc.to_json_bytes = lambda: _split_multiwaits(orig())
        _CACHE["nc"] = nc
    return _CACHE["nc"]


def _make_in_maps(x, Wq, Wk, Wv):
    bf16 = ml_dtypes.bfloat16
    wq = np.ascontiguousarray(Wq.astype(bf16))
    wk = np.ascontiguousarray(Wk.astype(bf16))
    wv = np.ascontiguousarray(Wv.astype(bf16))
    in_maps = []
    for i in range(B):
        xTb = np.ascontiguousarray(x[i].astype(bf16).T)  # [E, T]
        in_maps.append({"xT": xTb, "wq": wq, "wk": wk, "wv": wv})
    return in_maps


def kernel(x, Wq, Wk, Wv):
    from concourse.bass_utils import run_bass_kernel_spmd

    nc = _get_nc()
    in_maps = _make_in_maps(x, Wq, Wk, Wv)
    res = run_bass_kernel_spmd(nc, in_maps, core_ids=list(range(B)))
    out = np.stack([np.asarray(res.results[i]["out"]).T for i in range(B)])
    return np.ascontiguousarray(out.astype(np.float32))


# revision 9
# speedup vs baseline: 1.5541x; 1.5541x over previous
"""Trainium2 Bass kernel for nn_AttentionHead_37469294691171.

Single attention head with softmax over the QUERY axis (dim=1 of the
[B, T(query), S(key)] score tensor), causal mask, B=8, T=4096, E=2048, D=128.

Strategy: pure data-parallel over batch — one batch element per NeuronCore,
no collectives.  Per core, everything is computed in the *transposed* score
layout W[s, t] = (k_s . q_t) * scale so the query-axis softmax becomes a
free-axis reduction:

  1. host pre-packs x[b] -> xpk [NB, 128, E/128, 512] bf16 (one contiguous
     16KB-per-partition DMA per t-block) and weights -> [128, E/128 * D]
  2. qT/kT/vT [D, T] projections via PSUM-accumulated matmuls over E-chunks
     (softmax scale folded into qT, v transposed back to natural [s, d])
  3. for each 128-row s-tile j and 512-col t-block b >= diag: one matmul
     gives W tile [s=128, t=512]; additive -30000 mask on the diagonal
     128x128 sub-tile; ScalarE exp with fused accum_out produces the
     unnormalized P tile (kept resident in SBUF, bf16) and the per-key
     partial sums of the softmax denominator in one pass
  4. l = sum of partials, v' = v * (1/l) folded per-partition into v
     (emitted per-j as soon as row j's last tile is done)
  5. outT[d, t-block] = sum_j v'_j^T @ P_j  (PSUM-accumulated, N=512)
  6. host transposes outT back

Only the lower causal triangle is ever computed (both for scores and for
the output contraction).
"""

import numpy as np
import ml_dtypes

B, T, E, D = 8, 4096, 2048, 128
NBLK = 512            # t-block width
NCHUNK = E // 128     # 16 contraction chunks for the projections
NB = T // NBLK        # 8 t-blocks
NJ = T // 128         # 32 s-tiles
SCALE = float(D) ** -0.5
NEG = -30000.0        # additive mask; exp(-30000) == 0 in f32

_CACHE = {}


def _split_multiwaits(bir: bytes) -> bytes:
    """walrus accepts at most ONE embedded sync-wait per engine instruction.

    Tile's scheduler attaches every required wait to the consuming
    instruction's sync_info, which trips "Too many sync wait commands" in
    walrus codegen whenever an instruction joins events from 2+ engines.
    Hoist all but the last wait onto wait-only EventSemaphore instructions
    (the exact representation `engine.wait_ge()` uses in raw bass) placed
    immediately before the instruction in the same engine stream.  Each
    engine executes its stream in the Tile-scheduled order and waits only
    reference earlier-scheduled events, so this preserves semantics.
    """
    import json

    j = json.loads(bir)
    n = 0
    for fn in j["functions"]:
        for bb in fn["blocks"]:
            out = []
            for inst in bb["instructions"]:
                si = inst.get("sync_info")
                waits = (si or {}).get("on_wait") or []
                if len(waits) > 1 and inst.get("opcode") != "ISA":
                    for w in waits[:-1]:
                        n += 1
                        out.append({
                            "debug": inst.get("debug", 0),
                            "engine": inst["engine"],
                            "ins": [],
                            "outs": [],
                            "name": f"{inst['name']}-hw{n}",
                            "opcode": "EventSemaphore",
                            "sync_info": {"on_update": [], "on_wait": [w]},
                        })
                    si["on_wait"] = [waits[-1]]
                out.append(inst)
            bb["instructions"] = out
    return json.dumps(j).encode()


def _build():
    import concourse.bass as bass
    import concourse.mybir as mybir
    from concourse.tile import TileContext
    from concourse.masks import make_identity

    f32 = mybir.dt.float32
    bf16 = mybir.dt.bfloat16
    Alu = mybir.AluOpType

    nc = bass.Bass()
    # host-packed layouts (see _make_in_maps)
    xpk_d = nc.declare_dram_parameter("xpk", [NB, 128, NCHUNK, NBLK], bf16,
                                      isOutput=False)
    wq_d = nc.declare_dram_parameter("wq", [128, NCHUNK * D], bf16, isOutput=False)
    wk_d = nc.declare_dram_parameter("wk", [128, NCHUNK * D], bf16, isOutput=False)
    wv_d = nc.declare_dram_parameter("wv", [128, NCHUNK * D], bf16, isOutput=False)
    out_d = nc.declare_dram_parameter("out", [D, T], f32, isOutput=True)

    with TileContext(nc) as tc:
        with (
            tc.tile_pool(name="const", bufs=1) as constp,
            tc.tile_pool(name="big", bufs=1) as bigp,
            tc.tile_pool(name="pstore", bufs=1) as pp,
        ):
            ident = constp.tile([128, 128], bf16)
            make_identity(nc, ident[:])

            # trimask[p, c] = 0 where c >= p else NEG  (allowed: t >= s)
            trimask = constp.tile([128, 128], f32)
            nc.gpsimd.memset(trimask[:], 0.0)
            nc.gpsimd.affine_select(
                out=trimask[:],
                in_=trimask[:],
                compare_op=Alu.is_ge,
                fill=NEG,
                base=0,
                pattern=[[1, 128]],
                channel_multiplier=-1,
            )

            w_sb = {}
            for nm, wd in (("wq", wq_d), ("wk", wk_d), ("wv", wv_d)):
                t_ = constp.tile([128, NCHUNK, D], bf16, tag=f"w_{nm}")
                nc.sync.dma_start(t_[:], wd.rearrange("p (c d) -> p c d", c=NCHUNK))
                w_sb[nm] = t_

            qT = bigp.tile([128, T], bf16, tag="qT")
            kT = bigp.tile([128, T], bf16, tag="kT")
            v_nat = bigp.tile([128, T], bf16, tag="v")  # 32 tiles of [s=128, d=128]
            lpart = bigp.tile([128, NJ, NB], f32, tag="lpart")
            lsum = bigp.tile([128, NJ], f32, tag="lsum")
            linv = bigp.tile([128, NJ], f32, tag="linv")
            nc.vector.memset(lpart[:], 0.0)

            ptiles = [
                pp.tile([128, T - 128 * j], bf16, tag=f"P{j}", name=f"P{j}")
                for j in range(NJ)
            ]

            with (
                tc.tile_pool(name="xtb", bufs=2) as xtbp,
                tc.tile_pool(name="vtb", bufs=2) as vtbp,
                tc.tile_pool(name="ps_qkv", bufs=2, space="PSUM") as ps_qkv,
                tc.tile_pool(name="ps_vt", bufs=2, space="PSUM") as ps_vt,
                tc.tile_pool(name="ps_w", bufs=4, space="PSUM") as ps_w,
            ):
                for b in range(NB):
                    bs = slice(b * NBLK, (b + 1) * NBLK)
                    xtb = xtbp.tile([128, NCHUNK, NBLK], bf16, tag="xtb")
                    nc.sync.dma_start(xtb[:], xpk_d[b])

                    # ---- projections for this t-block ----
                    psq = ps_qkv.tile([128, NBLK], f32, tag="qkv")
                    for c in range(NCHUNK):
                        nc.tensor.matmul(psq[:], lhsT=w_sb["wq"][:, c, :],
                                         rhs=xtb[:, c, :],
                                         start=(c == 0), stop=(c == NCHUNK - 1))
                    nc.vector.tensor_scalar_mul(qT[:, bs], psq[:], SCALE)

                    psk = ps_qkv.tile([128, NBLK], f32, tag="qkv")
                    for c in range(NCHUNK):
                        nc.tensor.matmul(psk[:], lhsT=w_sb["wk"][:, c, :],
                                         rhs=xtb[:, c, :],
                                         start=(c == 0), stop=(c == NCHUNK - 1))
                    nc.vector.tensor_copy(kT[:, bs], psk[:])

                    psv = ps_qkv.tile([128, NBLK], f32, tag="qkv")
                    for c in range(NCHUNK):
                        nc.tensor.matmul(psv[:], lhsT=w_sb["wv"][:, c, :],
                                         rhs=xtb[:, c, :],
                                         start=(c == 0), stop=(c == NCHUNK - 1))
                    vtb = vtbp.tile([128, NBLK], bf16, tag="vtb")
                    nc.vector.tensor_copy(vtb[:], psv[:])
                    for i in range(4):
                        pst = ps_vt.tile([128, 128], bf16, tag="vt")
                        nc.tensor.transpose(pst[:], vtb[:, i * 128:(i + 1) * 128],
                                            ident[:])
                        j = 4 * b + i
                        nc.vector.tensor_copy(v_nat[:, j * 128:(j + 1) * 128], pst[:])

                    # ---- score stats rows ----
                    for j in range(4 * b + 4):
                        psw = ps_w.tile([128, NBLK], f32, tag="w")
                        nc.tensor.matmul(psw[:], lhsT=kT[:, j * 128:(j + 1) * 128],
                                         rhs=qT[:, bs], start=True, stop=True)
                        if j >= 4 * b:
                            off = (j - 4 * b) * 128
                            nc.vector.tensor_tensor(psw[:, off:off + 128],
                                                    psw[:, off:off + 128],
                                                    trimask[:], Alu.add)
                            src = psw[:, off:NBLK]
                            dst = ptiles[j][:, 0:NBLK - off]
                        else:
                            c0 = b * NBLK - 128 * j
                            src = psw[:]
                            dst = ptiles[j][:, c0:c0 + NBLK]
                        nc.scalar.activation(dst, src,
                                             mybir.ActivationFunctionType.Exp,
                                             accum_out=lpart[:, j, b:b + 1])
                        if b == NB - 1:
                            # row j is complete: softmax denominator and
                            # fold 1/l into v_j right away so the output
                            # pass can start while later rows still exp
                            nc.vector.tensor_reduce(
                                lsum[:, j:j + 1], lpart[:, j, :],
                                axis=mybir.AxisListType.X, op=Alu.add)
                            nc.vector.reciprocal(linv[:, j:j + 1],
                                                 lsum[:, j:j + 1])
                            nc.vector.tensor_scalar(
                                v_nat[:, j * 128:(j + 1) * 128],
                                v_nat[:, j * 128:(j + 1) * 128],
                                linv[:, j:j + 1], None, Alu.mult)

            # ---- output: outT[d, t-block] = sum_j v'_j^T @ P_j ----
            with (
                tc.tile_pool(name="outp", bufs=2) as outp,
                tc.tile_pool(name="ps_o", bufs=4, space="PSUM") as ps_o,
            ):
                for b in range(NB):
                    bs = slice(b * NBLK, (b + 1) * NBLK)
                    pso = ps_o.tile([128, NBLK], f32, tag="o")
                    nmm = 4 * b + 4
                    for j in range(nmm):
                        vpj = v_nat[:, j * 128:(j + 1) * 128]
                        last = (j == nmm - 1)
                        if j >= 4 * b:
                            off = (j - 4 * b) * 128
                            nc.tensor.matmul(pso[:, off:NBLK], lhsT=vpj,
                                             rhs=ptiles[j][:, 0:NBLK - off],
                                             start=(j == 0), stop=last)
                        else:
                            c0 = b * NBLK - 128 * j
                            nc.tensor.matmul(pso[:], lhsT=vpj,
                                             rhs=ptiles[j][:, c0:c0 + NBLK],
                                             start=(j == 0), stop=last)
                    ob = outp.tile([128, NBLK], f32, tag="ob")
                    nc.vector.tensor_copy(ob[:], pso[:])
                    nc.sync.dma_start(out_d[:, bs], ob[:])

    return nc


def _get_nc():
    if "nc" not in _CACHE:
        nc = _build()
        orig = nc.to_json_bytes
        nc.to_json_bytes = lambda: _split_multiwaits(orig())
        _CACHE["nc"] = nc
    return _CACHE["nc"]


def _make_in_maps(x, Wq, Wk, Wv):
    bf16 = ml_dtypes.bfloat16

    def pack_w(W):
        # [E, D] -> [128, NCHUNK*D] where row p holds chunks c at [c*D:(c+1)*D]
        # matching SBUF tile [128, NCHUNK, D] with lhsT chunk = W[c*128+p, :]
        return np.ascontiguousarray(
            W.astype(bf16).reshape(NCHUNK, 128, D).transpose(1, 0, 2).reshape(128, NCHUNK * D)
        )

    wq, wk, wv = pack_w(Wq), pack_w(Wk), pack_w(Wv)
    in_maps = []
    for i in range(B):
        # x[i]: [T, E] -> xT [E, T] -> xpk[b, p, c, t] = xT[c*128+p, b*512+t]
        xT = x[i].astype(bf16).T  # [E, T] view
        xpk = np.ascontiguousarray(
            xT.reshape(NCHUNK, 128, NB, NBLK).transpose(2, 1, 0, 3)
        )
        in_maps.append({"xpk": xpk, "wq": wq, "wk": wk, "wv": wv})
    return in_maps


def kernel(x, Wq, Wk, Wv):
    from concourse.bass_utils import run_bass_kernel_spmd

    nc = _get_nc()
    in_maps = _make_in_maps(x, Wq, Wk, Wv)
    res = run_bass_kernel_spmd(nc, in_maps, core_ids=list(range(B)))
    out = np.stack([np.asarray(res.results[i]["out"]).T for i in range(B)])
    return np.ascontiguousarray(out.astype(np.float32))


# revision 12
# speedup vs baseline: 1.6490x; 1.0611x over previous
"""Trainium2 Bass kernel for nn_AttentionHead_37469294691171.

Single attention head with softmax over the QUERY axis (dim=1 of the
[B, T(query), S(key)] score tensor), causal mask, B=8, T=4096, E=2048, D=128.

Strategy: pure data-parallel over batch — one batch element per NeuronCore,
no collectives.  Per core, everything is computed in the *transposed* score
layout W[s, t] = (k_s . q_t) * scale so the query-axis softmax becomes a
free-axis reduction:

  1. host pre-packs x[b] -> xpk [NB, 128, E/128, 512] bf16 (one contiguous
     16KB-per-partition DMA per t-block) and weights -> [128, E/128 * D]
  2. qT/kT/vT [D, T] projections via PSUM-accumulated matmuls over E-chunks
     (softmax scale folded into qT, v transposed back to natural [s, d])
  3. for each 128-row s-tile j and 512-col t-block b >= diag: one matmul
     gives W tile [s=128, t=512]; additive -30000 mask on the diagonal
     128x128 sub-tile; ScalarE exp with fused accum_out produces the
     unnormalized P tile (kept resident in SBUF, bf16) and the per-key
     partial sums of the softmax denominator in one pass
  4. l = sum of partials, v' = v * (1/l) folded per-partition into v
     (emitted per-j as soon as row j's last tile is done)
  5. outT[d, t-block] = sum_j v'_j^T @ P_j  (PSUM-accumulated, N=512)
  6. host transposes outT back

Only the lower causal triangle is ever computed (both for scores and for
the output contraction).
"""

import numpy as np
import ml_dtypes

B, T, E, D = 8, 4096, 2048, 128
NBLK = 512            # t-block width
NCHUNK = E // 128     # 16 contraction chunks for the projections
NB = T // NBLK        # 8 t-blocks
NJ = T // 128         # 32 s-tiles
SCALE = float(D) ** -0.5
NEG = -30000.0        # additive mask; exp(-30000) == 0 in f32

_CACHE = {}


def _split_multiwaits(bir: bytes) -> bytes:
    """walrus accepts at most ONE embedded sync-wait per engine instruction.

    Tile's scheduler attaches every required wait to the consuming
    instruction's sync_info, which trips "Too many sync wait commands" in
    walrus codegen whenever an instruction joins events from 2+ engines.
    Hoist all but the last wait onto wait-only EventSemaphore instructions
    (the exact representation `engine.wait_ge()` uses in raw bass) placed
    immediately before the instruction in the same engine stream.  Each
    engine executes its stream in the Tile-scheduled order and waits only
    reference earlier-scheduled events, so this preserves semantics.
    """
    import json

    j = json.loads(bir)
    n = 0
    for fn in j["functions"]:
        for bb in fn["blocks"]:
            out = []
            for inst in bb["instructions"]:
                si = inst.get("sync_info")
                waits = (si or {}).get("on_wait") or []
                if len(waits) > 1 and inst.get("opcode") != "ISA":
                    for w in waits[:-1]:
                        n += 1
                        out.append({
                            "debug": inst.get("debug", 0),
                            "engine": inst["engine"],
                            "ins": [],
                            "outs": [],
                            "name": f"{inst['name']}-hw{n}",
                            "opcode": "EventSemaphore",
                            "sync_info": {"on_update": [], "on_wait": [w]},
                        })
                    si["on_wait"] = [waits[-1]]
                out.append(inst)
            bb["instructions"] = out
    return json.dumps(j).encode()


def _build():
    import concourse.bass as bass
    import concourse.mybir as mybir
    from concourse.tile import TileContext
    from concourse.masks import make_identity

    f32 = mybir.dt.float32
    bf16 = mybir.dt.bfloat16
    Alu = mybir.AluOpType

    nc = bass.Bass()
    # host-packed layouts (see _make_in_maps)
    xpk_d = nc.declare_dram_parameter("xpk", [NB, 128, NCHUNK, NBLK], bf16,
                                      isOutput=False)
    wq_d = nc.declare_dram_parameter("wq", [128, NCHUNK * D], bf16, isOutput=False)
    wk_d = nc.declare_dram_parameter("wk", [128, NCHUNK * D], bf16, isOutput=False)
    wv_d = nc.declare_dram_parameter("wv", [128, NCHUNK * D], bf16, isOutput=False)
    out_d = nc.declare_dram_parameter("out", [D, T], f32, isOutput=True)

    with TileContext(nc) as tc:
        with (
            tc.tile_pool(name="const", bufs=1) as constp,
            tc.tile_pool(name="big", bufs=1) as bigp,
            tc.tile_pool(name="pstore", bufs=1) as pp,
        ):
            ident = constp.tile([128, 128], bf16)
            make_identity(nc, ident[:])

            # trimask[p, c] = 0 where c >= p else NEG  (allowed: t >= s)
            trimask = constp.tile([128, 128], f32)
            nc.gpsimd.memset(trimask[:], 0.0)
            nc.gpsimd.affine_select(
                out=trimask[:],
                in_=trimask[:],
                compare_op=Alu.is_ge,
                fill=NEG,
                base=0,
                pattern=[[1, 128]],
                channel_multiplier=-1,
            )

            # weights go on the Activation HWDGE queue so they don't queue
            # behind the first x-block loads on the SP queue
            w_sb = {}
            for nm, wd in (("wq", wq_d), ("wk", wk_d), ("wv", wv_d)):
                t_ = constp.tile([128, NCHUNK, D], bf16, tag=f"w_{nm}")
                nc.scalar.dma_start(t_[:], wd.rearrange("p (c d) -> p c d", c=NCHUNK))
                w_sb[nm] = t_

            qT = bigp.tile([128, T], bf16, tag="qT")
            kT = bigp.tile([128, T], bf16, tag="kT")
            v_nat = bigp.tile([128, T], bf16, tag="v")  # 32 tiles of [s=128, d=128]
            lpart = bigp.tile([128, NJ, NB], f32, tag="lpart")
            lsum = bigp.tile([128, NJ], f32, tag="lsum")
            linv = bigp.tile([128, NJ], f32, tag="linv")
            nc.vector.memset(lpart[:], 0.0)

            ptiles = [
                pp.tile([128, T - 128 * j], bf16, tag=f"P{j}", name=f"P{j}")
                for j in range(NJ)
            ]

            with (
                tc.tile_pool(name="xtb", bufs=2) as xtbp,
                tc.tile_pool(name="vtb", bufs=2) as vtbp,
                tc.tile_pool(name="ps_qkv", bufs=2, space="PSUM") as ps_qkv,
                tc.tile_pool(name="ps_vt", bufs=2, space="PSUM") as ps_vt,
                tc.tile_pool(name="ps_w", bufs=4, space="PSUM") as ps_w,
            ):
                for b in range(NB):
                    bs = slice(b * NBLK, (b + 1) * NBLK)
                    xtb = xtbp.tile([128, NCHUNK, NBLK], bf16, tag="xtb")
                    if b == 0:
                        # split the first load so the projections can start
                        # on the leading chunks while the rest streams in
                        for q in range(4):
                            nc.sync.dma_start(xtb[:, q * 4:(q + 1) * 4, :],
                                              xpk_d[b, :, q * 4:(q + 1) * 4, :])
                    else:
                        nc.sync.dma_start(xtb[:], xpk_d[b])

                    # ---- projections for this t-block ----
                    psq = ps_qkv.tile([128, NBLK], f32, tag="qkv")
                    for c in range(NCHUNK):
                        nc.tensor.matmul(psq[:], lhsT=w_sb["wq"][:, c, :],
                                         rhs=xtb[:, c, :],
                                         start=(c == 0), stop=(c == NCHUNK - 1))
                    nc.vector.tensor_scalar_mul(qT[:, bs], psq[:], SCALE)

                    psk = ps_qkv.tile([128, NBLK], f32, tag="qkv")
                    for c in range(NCHUNK):
                        nc.tensor.matmul(psk[:], lhsT=w_sb["wk"][:, c, :],
                                         rhs=xtb[:, c, :],
                                         start=(c == 0), stop=(c == NCHUNK - 1))
                    nc.vector.tensor_copy(kT[:, bs], psk[:])

                    psv = ps_qkv.tile([128, NBLK], f32, tag="qkv")
                    for c in range(NCHUNK):
                        nc.tensor.matmul(psv[:], lhsT=w_sb["wv"][:, c, :],
                                         rhs=xtb[:, c, :],
                                         start=(c == 0), stop=(c == NCHUNK - 1))
                    vtb = vtbp.tile([128, NBLK], bf16, tag="vtb")
                    nc.vector.tensor_copy(vtb[:], psv[:])
                    for i in range(4):
                        pst = ps_vt.tile([128, 128], bf16, tag="vt")
                        nc.tensor.transpose(pst[:], vtb[:, i * 128:(i + 1) * 128],
                                            ident[:])
                        j = 4 * b + i
                        nc.vector.tensor_copy(v_nat[:, j * 128:(j + 1) * 128], pst[:])

                    # ---- score stats rows ----
                    for j in range(4 * b + 4):
                        psw = ps_w.tile([128, NBLK], f32, tag="w")
                        if j >= 4 * b:
                            # diagonal: only cols [off:512] are causal-allowed
                            off = (j - 4 * b) * 128
                            nc.tensor.matmul(
                                psw[:, off:NBLK],
                                lhsT=kT[:, j * 128:(j + 1) * 128],
                                rhs=qT[:, b * NBLK + off:(b + 1) * NBLK],
                                start=True, stop=True)
                            nc.vector.tensor_tensor(psw[:, off:off + 128],
                                                    psw[:, off:off + 128],
                                                    trimask[:], Alu.add)
                            src = psw[:, off:NBLK]
                            dst = ptiles[j][:, 0:NBLK - off]
                        else:
                            nc.tensor.matmul(
                                psw[:], lhsT=kT[:, j * 128:(j + 1) * 128],
                                rhs=qT[:, bs], start=True, stop=True)
                            c0 = b * NBLK - 128 * j
                            src = psw[:]
                            dst = ptiles[j][:, c0:c0 + NBLK]
                        if (j + b) % 2 == 0:
                            # row-sum on DVE: balances the ScalarE exp load
                            # (ACT's accum_out costs a separate 318ns
                            # ACTIVATION_READ_ACCUMULATOR per tile)
                            nc.scalar.activation(dst, src,
                                                 mybir.ActivationFunctionType.Exp)
                            nc.vector.tensor_reduce(
                                lpart[:, j, b:b + 1], dst,
                                axis=mybir.AxisListType.X, op=Alu.add)
                        else:
                            nc.scalar.activation(dst, src,
                                                 mybir.ActivationFunctionType.Exp,
                                                 accum_out=lpart[:, j, b:b + 1])
                        if b == NB - 1:
                            # row j is complete: softmax denominator and
                            # fold 1/l into v_j right away so the output
                            # pass can start while later rows still exp
                            nc.vector.tensor_reduce(
                                lsum[:, j:j + 1], lpart[:, j, :],
                                axis=mybir.AxisListType.X, op=Alu.add)
                            nc.vector.reciprocal(linv[:, j:j + 1],
                                                 lsum[:, j:j + 1])
                            nc.vector.tensor_scalar(
                                v_nat[:, j * 128:(j + 1) * 128],
                                v_nat[:, j * 128:(j + 1) * 128],
                                linv[:, j:j + 1], None, Alu.mult)

            # ---- output: outT[d, t-block] = sum_j v'_j^T @ P_j ----
            with (
                tc.tile_pool(name="outp", bufs=2) as outp,
                tc.tile_pool(name="ps_o", bufs=4, space="PSUM") as ps_o,
            ):
                for b in range(NB):
                    bs = slice(b * NBLK, (b + 1) * NBLK)
                    pso = ps_o.tile([128, NBLK], f32, tag="o")
                    nmm = 4 * b + 4
                    for j in range(nmm):
                        vpj = v_nat[:, j * 128:(j + 1) * 128]
                        last = (j == nmm - 1)
                        if j >= 4 * b:
                            off = (j - 4 * b) * 128
                            nc.tensor.matmul(pso[:, off:NBLK], lhsT=vpj,
                                             rhs=ptiles[j][:, 0:NBLK - off],
                                             start=(j == 0), stop=last)
                        else:
                            c0 = b * NBLK - 128 * j
                            nc.tensor.matmul(pso[:], lhsT=vpj,
                                             rhs=ptiles[j][:, c0:c0 + NBLK],
                                             start=(j == 0), stop=last)
                    ob = outp.tile([128, NBLK], f32, tag="ob")
                    nc.vector.tensor_copy(ob[:], pso[:])
                    nc.sync.dma_start(out_d[:, bs], ob[:])

    return nc


def _get_nc():
    if "nc" not in _CACHE:
        nc = _build()
        orig = nc.to_json_bytes
        nc.to_json_bytes = lambda: _split_multiwaits(orig())
        _CACHE["nc"] = nc
    return _CACHE["nc"]


def _make_in_maps(x, Wq, Wk, Wv):
    bf16 = ml_dtypes.bfloat16

    def pack_w(W):
        # [E, D] -> [128, NCHUNK*D] where row p holds chunks c at [c*D:(c+1)*D]
        # matching SBUF tile [128, NCHUNK, D] with lhsT chunk = W[c*128+p, :]
        return np.ascontiguousarray(
            W.astype(bf16).reshape(NCHUNK, 128, D).transpose(1, 0, 2).reshape(128, NCHUNK * D)
        )

    wq, wk, wv = pack_w(Wq), pack_w(Wk), pack_w(Wv)
    in_maps = []
    for i in range(B):
        # x[i]: [T, E] -> xT [E, T] -> xpk[b, p, c, t] = xT[c*128+p, b*512+t]
        xT = x[i].astype(bf16).T  # [E, T] view
        xpk = np.ascontiguousarray(
            xT.reshape(NCHUNK, 128, NB, NBLK).transpose(2, 1, 0, 3)
        )
        in_maps.append({"xpk": xpk, "wq": wq, "wk": wk, "wv": wv})
    return in_maps


def kernel(x, Wq, Wk, Wv):
    from concourse.bass_utils import run_bass_kernel_spmd

    nc = _get_nc()
    in_maps = _make_in_maps(x, Wq, Wk, Wv)
    res = run_bass_kernel_spmd(nc, in_maps, core_ids=list(range(B)))
    out = np.stack([np.asarray(res.results[i]["out"]).T for i in range(B)])
    return np.ascontiguousarray(out.astype(np.float32))
